# revision 30
# baseline (speedup 1.0000x reference)
"""Trainium2 Bass kernel for nn_DenoisingNet_MLP_3 (LISTA denoiser, 2 stages).

Strategy: 8 cores = 2 samples x 4 patch-row chunks. The device runs the heavy
per-token pipeline (thr/Wg MLPs, y, 5 LISTA iterations, x_pred). The three
per-token MLPs run in fp8 (e4m3 DoubleRow matmuls, 2x PE rate) with host-side
calibration: inputs centered (uf-0.5, h-mean from the 112 ext tokens), per
tensor pow2 scales folded into activation scale/bias APs. The LISTA loop and
dictionary products stay fp32r. Elementwise work is spread across Scalar
(relu/copies), Vector (soft-threshold DVE, PSUM adds) and Pool (SBUF adds,
fp8 casts). The host runs the tiny per-sample ops (sd-MLP/CBAM -> Dcat),
unfold slicing and the overlap-add fold. One compiled NEFF serves both stages.
"""
import numpy as np
import ml_dtypes
import concourse.bass as bass
import concourse.bacc as bacc
import concourse.mybir as mybir
import concourse.tile as tile
from concourse.bass_utils import run_bass_kernel_spmd

fp32 = mybir.dt.float32
fp32r = mybir.dt.float32r
bf16 = mybir.dt.bfloat16
f8e4 = mybir.dt.float8e4
Alu = mybir.AluOpType
Act = mybir.ActivationFunctionType
DR = mybir.MatmulPerfMode.DoubleRow
F8NP = ml_dtypes.float8_e4m3
BF16NP = ml_dtypes.bfloat16

# ---- fused soft-threshold custom DVE op: out = sign(x) * relu(|x| - l) ----
import concourse.dve_ops as _dvo
from concourse.dve_spec import Spec as _Spec, Src0 as _S0, Src1 as _S1, Zero as _Z0, \
    C0 as _C0, C1 as _C1, C2 as _C2, relu as _relu, maxx as _maxx, lower as _lower
from concourse.dve_uop import DveOpSpec as _DveOpSpec


def _soft_ref(in0, in1, s0, s1, imm2):
    x = in0.astype(np.float32)
    return np.sign(x) * np.maximum(np.abs(x) - in1.astype(np.float32), 0.0)


def _rc_ref(in0, in1, s0, s1, imm2):
    x = in0.astype(np.float32)
    return (np.maximum(x + s0, 0.0) - s1) * imm2


def _reg_op(name, spec, rd1):
    op = _dvo.DveOp(name, spec, subdim=False, uops_sha={})
    if name not in _dvo._SUB_OPCODE_FOR_NAME:
        _dvo.OPS.append(op)
        _dvo._SUB_OPCODE_FOR_NAME[name] = _dvo._CUSTOM_DVE_ROW_BASE + len(_dvo.OPS) - 1
    for _ver in ("v3", "v4"):
        try:
            _sp = _DveOpSpec(name=name, opcode=_dvo.get_dve_sub_opcode(name),
                             uops=_lower(spec, ver=_ver), rd1_en=rd1)
            op.uops_sha[_ver] = _sp.sha(_ver)
        except Exception:
            pass
    return op


_SOFT_SPEC = _Spec(body=((_S0 > _Z0) - (_S0 < _Z0)) * _relu(_maxx(_S0, _Z0 - _S0) - _S1),
                   reference=_soft_ref)
SOFT_SHRINK_ANT = _reg_op("SOFT_SHRINK_ANT", _SOFT_SPEC, True)
_RC_SPEC = _Spec(body=(_relu(_S0 + _C0) - _C1) * _C2, reference=_rc_ref)
RELU_CENTER_ANT = _reg_op("RELU_CENTER_ANT", _RC_SPEC, False)

KP = 16            # patch size
P2 = 256           # patch features
DD = 624
PR = 113           # stride-1 patch grid is 113x113
L_FULL = PR * PR               # 12769 tokens per sample
T0S = [0, 3192, 6384, 9576]    # first token per core chunk (1-token overlaps)
LC = 3193                      # tokens per core
TILE_NV = [457, 456, 456, 456, 456, 456, 456]   # 7 token tiles per core
D_SZ = [128, 128, 128, 128, 112]
T_LISTA = 5
SU = 8.0                       # uf fp8 pre-scale: (uf - 0.5) * SU
C2IMM = 2.0 ** -6              # fixed post-scale of the fused relu-center DVE op
N3 = {"pd": 512, "lam": 112, "w": 256}
NB3 = {"pd": 4, "lam": 1, "w": 2}
# cst blob column offsets: b1s[8] mu1s[8] b2s[4] mu2s[4] b3s[nb3] k1 k2 k3
CST_B1, CST_MU1, CST_B2, CST_MU2, CST_B3 = 0, 8, 16, 20, 24
CST_K = {p: 24 + NB3[p] for p in N3}
CST_NCOL = {p: 24 + NB3[p] + 3 for p in N3}
W8_F = {p: 2048 + 4096 + 4 * N3[p] for p in N3}

_NC_CACHE: dict = {}
_WBLOB_CACHE: dict = {}


def _p2(x):
    return float(2.0 ** np.floor(np.log2(x)))


# --------------------------------------------------------------------------
# device program
# --------------------------------------------------------------------------

def _build(c_val: float):
    nc = bacc.Bacc("TRN2", target_bir_lowering=False, debug=False, num_devices=8)

    FT = sum(2 * (nv + nv % 2) for nv in TILE_NV)
    ufall_d = nc.dram_tensor("ufall", [128, FT], fp32r, kind="ExternalInput")
    w8d, cstd = {}, {}
    for pre in ("pd", "lam", "w"):
        for li, fl in ((1, 2048), (2, 4096), (3, 4 * N3[pre])):
            w8d[(pre, li)] = nc.dram_tensor(f"{pre}w8_{li}", [128, fl], f8e4, kind="ExternalInput")
        cstd[pre] = nc.dram_tensor(f"{pre}cst", [128, CST_NCOL[pre]], fp32, kind="ExternalInput")
    dcat_d = nc.dram_tensor("dcat_b", [128, 2 * DD], fp32r, kind="ExternalInput")
    dcatT_d = nc.dram_tensor("dcatT_b", [128, 5 * 256], fp32r, kind="ExternalInput")
    eye_d = nc.dram_tensor("eye_b", [128, 128], fp32r, kind="ExternalInput")
    px_o = nc.dram_tensor("px_o", [256, LC], bf16, kind="ExternalOutput")
    wg_o = nc.dram_tensor("wg_o", [256, LC], bf16, kind="ExternalOutput")

    inv_c = float(1.0 / c_val)

    with tile.TileContext(nc) as tc:
        with (
            tc.tile_pool(name="fx", bufs=1) as fx,
            tc.tile_pool(name="wk", bufs=1) as wk,
            tc.tile_pool(name="pp", bufs=1, space="PSUM") as pp,
        ):
            # ---- persistent loads: csts first (tiny, scalar queue); weight
            # layers stream on the gpsimd queue in exact first-use order so
            # compute starts as soon as the first 256KB lands ----
            ws, cs = {}, {}
            for pre in ("pd", "lam", "w"):
                ct = fx.tile([128, CST_NCOL[pre]], fp32, name=f"sb_{pre}cst")
                nc.scalar.dma_start(
                    out=ct[:],
                    in_=bass.AP(cstd[pre], 0, [[CST_NCOL[pre], 128], [1, CST_NCOL[pre]]]),
                )
                cs[pre] = ct
            for pre in ("pd", "lam", "w"):
                for li, fl in ((1, 2048), (2, 4096), (3, 4 * N3[pre])):
                    t = fx.tile([128, fl], f8e4, name=f"sb_{pre}w8_{li}")
                    nc.gpsimd.dma_start(
                        out=t[:], in_=bass.AP(w8d[(pre, li)], 0, [[fl, 128], [1, fl]])
                    )
                    ws[(pre, li)] = t
            dcat = fx.tile([128, 2 * DD], fp32r, name="sb_dcat")
            nc.gpsimd.dma_start(
                out=dcat[:], in_=bass.AP(dcat_d, 0, [[2 * DD, 128], [1, 2 * DD]])
            )
            dcatT = fx.tile([128, 5 * 256], fp32r, name="sb_dcatT")
            nc.gpsimd.dma_start(
                out=dcatT[:], in_=bass.AP(dcatT_d, 0, [[5 * 256, 128], [1, 5 * 256]])
            )
            eye = fx.tile([128, 128], fp32r, name="sb_eye")
            nc.gpsimd.dma_start(
                out=eye[:], in_=bass.AP(eye_d, 0, [[128, 128], [1, 128]])
            )
            # dcatN = -dcat/c derived on-device instead of a 640KB upload
            dcatN = fx.tile([128, 2 * DD], fp32r, name="sb_dcatN")
            nc.scalar.mul(dcatN[:], dcat[:], -inv_c)

            # ---- per-tile geometry ----
            def geom(t):
                Nv = TILE_NV[t]
                return None, None, Nv, Nv + (Nv % 2), sum(TILE_NV[:t])

            def ufbase(t):
                return sum(2 * (nv + nv % 2) for nv in TILE_NV[:t])

            st = [dict() for _ in TILE_NV]   # per-tile live handles

            # ---- prologue pieces (overlappable) ----
            def p_unfold(t):
                _, _, Nv, N, tok0 = geom(t)
                uf = wk.tile([128, 2 * N], fp32r, name=f"uf{t}", tag="uf", bufs=2)
                uf8 = wk.tile([128, 2 * N], f8e4, name=f"uf8_{t}", tag="uf8", bufs=2)
                nc.sync.dma_start(
                    out=uf[:],
                    in_=bass.AP(ufall_d, ufbase(t), [[FT, 128], [1, 2 * N]]),
                )
                # uf8 = fp8((uf - 0.5) * SU), one cheap DVE pass
                nc.vector.tensor_scalar(
                    uf8[:], uf[:].bitcast(fp32), 0.5, SU, Alu.subtract, Alu.mult
                )
                st[t]["uf"] = uf
                st[t]["uf8"] = uf8
                st[t]["thr"] = wk.tile([128, 5 * N], fp32, name=f"thr{t}", tag="thr", bufs=2)
                st[t]["wg"] = wk.tile([128, 2 * N], bf16, name=f"wg{t}", tag="wg", bufs=2)

            def p_l1(t, pre):
                _, _, _, N, _ = geom(t)
                uf8 = st[t]["uf8"][:].rearrange("p (k n) -> p k n", k=2)
                ct = cs[pre]
                h18 = wk.tile([128, 8 * N], f8e4, name=f"h18_{t}_{pre}", tag="h18", bufs=2)
                w1v = ws[(pre, 1)][:].rearrange("p (j m) -> p j m", j=2)
                for g in range(8):
                    ps1 = pp.tile([128, N], fp32, name=f"ps1_{t}_{pre}_{g}", tag="ps", bufs=8)
                    nc.tensor.matmul(
                        ps1[:], w1v[:, :, g * 128 : g * 128 + 128], uf8,
                        start=True, stop=True, perf_mode=DR,
                    )
                    if pre == "w":
                        # uncentered: h18 = relu(psum*k1 + s1*b1c) directly (Scalar)
                        nc.scalar.activation(
                            h18[:, g * N : (g + 1) * N], ps1[:], Act.Relu,
                            bias=ct[:, CST_B1 + g : CST_B1 + g + 1],
                            scale=ct[:, CST_K[pre] : CST_K[pre] + 1],
                        )
                    else:
                        # fused relu+center+cast on Vector: (relu(ps+a1*b1c)-a1*mu1)*C2
                        nc.vector._custom_dve(
                            RELU_CENTER_ANT, out=h18[:, g * N : (g + 1) * N],
                            in0=ps1[:],
                            s0=ct[:, CST_B1 + g : CST_B1 + g + 1],
                            s1=ct[:, CST_MU1 + g : CST_MU1 + g + 1],
                            imm2=C2IMM,
                        )
                st[t][f"h18_{pre}"] = h18

            def p_l2(t, pre):
                _, _, _, N, _ = geom(t)
                h18 = st[t][f"h18_{pre}"][:].rearrange("p (k n) -> p k n", k=8)
                ct = cs[pre]
                h28 = wk.tile([128, 4 * N], f8e4, name=f"h28_{t}_{pre}", tag="h28", bufs=2)
                w2v = ws[(pre, 2)][:].rearrange("p (j2 j m) -> p j2 j m", j2=4, j=2)
                for g in range(4):
                    ps2 = pp.tile([128, N], fp32, name=f"ps2_{t}_{pre}_{g}", tag="ps", bufs=8)
                    for j2 in range(4):
                        nc.tensor.matmul(
                            ps2[:], w2v[:, j2, :, g * 128 : g * 128 + 128],
                            h18[:, 2 * j2 : 2 * j2 + 2, :],
                            start=(j2 == 0), stop=(j2 == 3), perf_mode=DR,
                        )
                    if pre == "w":
                        nc.scalar.activation(
                            h28[:, g * N : (g + 1) * N], ps2[:], Act.Relu,
                            bias=ct[:, CST_B2 + g : CST_B2 + g + 1],
                            scale=ct[:, CST_K[pre] + 1 : CST_K[pre] + 2],
                        )
                    else:
                        nc.vector._custom_dve(
                            RELU_CENTER_ANT, out=h28[:, g * N : (g + 1) * N],
                            in0=ps2[:],
                            s0=ct[:, CST_B2 + g : CST_B2 + g + 1],
                            s1=ct[:, CST_MU2 + g : CST_MU2 + g + 1],
                            imm2=C2IMM,
                        )
                st[t][f"h28_{pre}"] = h28

            def p_l3(t, pre):
                _, _, _, N, _ = geom(t)
                n3 = N3[pre]
                h28 = st[t][f"h28_{pre}"][:].rearrange("p (k n) -> p k n", k=4)
                ct = cs[pre]
                thr, wg = st[t]["thr"], st[t]["wg"]
                w3v = ws[(pre, 3)][:].rearrange("p (j2 j m) -> p j2 j m", j2=2, j=2)
                for mo in range(NB3[pre]):
                    sz = min(128, n3 - mo * 128)
                    ps3 = pp.tile([128, N], fp32, name=f"ps3_{t}_{pre}_{mo}", tag="ps", bufs=8)
                    for j2 in range(2):
                        nc.tensor.matmul(
                            ps3[0:sz], w3v[:, j2, :, mo * 128 : mo * 128 + sz],
                            h28[:, 2 * j2 : 2 * j2 + 2, :],
                            start=(j2 == 0), stop=(j2 == 1), perf_mode=DR,
                        )
                    kap = ct[:, CST_K[pre] + 2 : CST_K[pre] + 3]
                    if pre == "pd":
                        nc.scalar.activation(
                            thr[:, mo * N : (mo + 1) * N], ps3[:], Act.Identity,
                            bias=ct[:, CST_B3 + mo : CST_B3 + mo + 1], scale=kap,
                        )
                    elif pre == "lam":
                        nc.scalar.activation(
                            thr[0:112, 4 * N : 5 * N], ps3[0:112], Act.Identity,
                            bias=ct[0:112, CST_B3 : CST_B3 + 1], scale=kap[0:112],
                        )
                    else:
                        nc.scalar.activation(
                            wg[:, mo * N : (mo + 1) * N], ps3[:], Act.Sigmoid,
                            bias=ct[:, CST_B3 + mo : CST_B3 + mo + 1], scale=kap,
                        )

            def p_y(t):
                _, _, _, N, _ = geom(t)
                uf, thr = st[t]["uf"], st[t]["thr"]
                yc = wk.tile([128, 5 * N], fp32r, name=f"yc{t}", tag="yc", bufs=2)
                z = wk.tile([128, 5 * N], fp32r, name=f"z{t}_0", tag="z", bufs=2)
                for mc in range(5):
                    sz = D_SZ[mc]
                    d0 = 128 * mc
                    psy = pp.tile([128, N], fp32, name=f"psy_{t}_{mc}", tag="ps", bufs=8)
                    nc.tensor.matmul(
                        psy[0:sz], dcat[:, d0 : d0 + sz], uf[:, 0:N], start=True, stop=False
                    )
                    nc.tensor.matmul(
                        psy[0:sz], dcat[:, DD + d0 : DD + d0 + sz], uf[:, N : 2 * N],
                        start=False, stop=True,
                    )
                    nc.scalar.mul(yc[0:sz, mc * N : (mc + 1) * N], psy[0:sz], inv_c)
                    nc.vector._custom_dve(
                        SOFT_SHRINK_ANT,
                        out=z[0:sz, mc * N : (mc + 1) * N],
                        in0=psy[0:sz], in1=thr[0:sz, mc * N : (mc + 1) * N],
                    )
                st[t]["yc"] = yc
                st[t]["z"] = z

            def prologue_pieces(t):
                yield from (
                    lambda: p_unfold(t),
                    lambda: p_l1(t, "pd"), lambda: p_l2(t, "pd"), lambda: p_l3(t, "pd"),
                    lambda: p_l1(t, "lam"), lambda: p_l2(t, "lam"), lambda: p_l3(t, "lam"),
                    lambda: p_l1(t, "w"), lambda: p_l2(t, "w"), lambda: p_l3(t, "w"),
                    lambda: p_y(t),
                )

            def lista_iter_a(t, it):
                _, _, _, N, _ = geom(t)
                yc, z = st[t]["yc"], st[t]["z"]
                # zt = z + yc, one coalesced DVE op; psg (which only needs z)
                # covers its latency before iter_b's I@zt consumes it
                zt = wk.tile([128, 5 * N], fp32r, name=f"zt{t}_{it}", tag="zt", bufs=2)
                nc.vector.tensor_tensor(
                    zt[:], z[:].bitcast(fp32), yc[:].bitcast(fp32), Alu.add
                )
                # G = Dcat @ z   [256, N]
                g = wk.tile([128, 2 * N], fp32r, name=f"g{t}_{it}", tag="g", bufs=2)
                for fc in range(2):
                    psg = pp.tile([128, N], fp32, name=f"psg_{t}_{it}_{fc}", tag="ps", bufs=8)
                    for kc in range(5):
                        szk = D_SZ[kc]
                        nc.tensor.matmul(
                            psg[:],
                            dcatT[0:szk, kc * 256 + fc * 128 : kc * 256 + fc * 128 + 128],
                            z[0:szk, kc * N : (kc + 1) * N],
                            start=(kc == 0), stop=(kc == 4),
                        )
                    nc.scalar.copy(g[:, fc * N : (fc + 1) * N], psg[:])
                st[t]["zt"] = zt
                st[t]["g"] = g

            def lista_iter_b(t, it):
                _, _, _, N, _ = geom(t)
                thr = st[t]["thr"]
                zt, g = st[t]["zt"], st[t]["g"]
                zn = wk.tile([128, 5 * N], fp32r, name=f"z{t}_{it + 1}", tag="z", bufs=2)
                for mc in range(5):
                    sz = D_SZ[mc]
                    d0 = 128 * mc
                    ps = pp.tile([128, N], fp32, name=f"psl_{t}_{it}_{mc}", tag="ps", bufs=8)
                    nc.tensor.matmul(
                        ps[0:sz], eye[0:sz, 0:sz], zt[0:sz, mc * N : (mc + 1) * N],
                        start=True, stop=False,
                    )
                    for fc in range(2):
                        nc.tensor.matmul(
                            ps[0:sz],
                            dcatN[:, fc * DD + d0 : fc * DD + d0 + sz],
                            g[:, fc * N : (fc + 1) * N],
                            start=False, stop=(fc == 1),
                        )
                    nc.vector._custom_dve(
                        SOFT_SHRINK_ANT,
                        out=zn[0:sz, mc * N : (mc + 1) * N],
                        in0=ps[0:sz], in1=thr[0:sz, mc * N : (mc + 1) * N],
                    )
                st[t]["z"] = zn

            def emit_xp(t):
                _, _, Nv, N, tok0 = geom(t)
                z, wg = st[t]["z"], st[t]["wg"]
                for fc in range(2):
                    psx = pp.tile([128, N], fp32, name=f"psx_{t}_{fc}", tag="ps", bufs=8)
                    for kc in range(5):
                        szk = D_SZ[kc]
                        nc.tensor.matmul(
                            psx[:],
                            dcatT[0:szk, kc * 256 + fc * 128 : kc * 256 + fc * 128 + 128],
                            z[0:szk, kc * N : (kc + 1) * N],
                            start=(kc == 0), stop=(kc == 4),
                        )
                    nc.vector.tensor_scalar(psx[:], psx[:], 0.0, 1.0, Alu.max, Alu.min)
                    px = wk.tile([128, N], bf16, name=f"px{t}_{fc}", tag="px", bufs=2)
                    nc.vector.tensor_tensor(px[:], psx[:], wg[:, fc * N : (fc + 1) * N], Alu.mult)
                    nc.scalar.dma_start(
                        out=bass.AP(px_o, fc * 128 * LC + tok0, [[LC, 128], [1, Nv]]),
                        in_=px[:, 0:Nv],
                    )
                    nc.scalar.dma_start(
                        out=bass.AP(wg_o, fc * 128 * LC + tok0, [[LC, 128], [1, Nv]]),
                        in_=wg[:, fc * N : fc * N + Nv],
                    )

            # ---- driver: software-pipeline tiles; next-tile prologue pieces
            # are slotted at BOTH half-iteration boundaries so the PE always
            # has independent work while g-copies / softs drain ----
            n_tiles = len(TILE_NV)
            for piece in prologue_pieces(0):
                piece()
            n_slots = 2 * T_LISTA
            for t in range(n_tiles):
                nxt = list(prologue_pieces(t + 1)) if t + 1 < n_tiles else []
                slots = [nxt[(len(nxt) * i) // n_slots : (len(nxt) * (i + 1)) // n_slots]
                         for i in range(n_slots)]
                for it in range(T_LISTA):
                    lista_iter_a(t, it)
                    for piece in slots[2 * it]:
                        piece()
                    lista_iter_b(t, it)
                    for piece in slots[2 * it + 1]:
                        piece()
                emit_xp(t)

    nc.compile()
    return nc


# --------------------------------------------------------------------------
# host-side small ops (per sample): ext -> sd MLP -> CBAM -> Dcat
# --------------------------------------------------------------------------

def _host_sd(img2d, p, c_val):
    # ext: stride-8 unfold, every 2nd patch, first 112   [112, 256]
    ext = np.empty((112, 256), np.float32)
    for tt in range(112):
        ir, ic = divmod(2 * tt, 15)
        ext[tt] = img2d[8 * ir : 8 * ir + 16, 8 * ic : 8 * ic + 16].reshape(256)
    h = ext
    for wname, bname in (("s1w", "s1b"), ("s2w", "s2b"), ("s3w", "s3b")):
        h = np.maximum(h @ p[wname] + p[bname], 0.0, dtype=np.float32)
    sd = (h @ p["s4w"] + p["s4b"]).astype(np.float32)          # [112, 256]
    nrm = np.maximum(np.linalg.norm(sd, axis=-1, keepdims=True), 1e-12)
    sd = (sd / nrm).astype(np.float32)
    v = sd.T.reshape(256, 8, 14)                                # channels, 8x14
    def camlp(vec):
        return np.maximum(vec @ p["caw1"], 0.0) @ p["caw2"]
    ca = 1.0 / (1.0 + np.exp(-(camlp(v.mean(axis=(1, 2))) + camlp(v.max(axis=(1, 2))))))
    v = (v * ca[:, None, None]).astype(np.float32)
    s2 = np.stack([v.mean(axis=0), v.max(axis=0)])              # [2, 8, 14]
    pad = np.zeros((2, 14, 20), np.float32)
    pad[:, 3:11, 3:17] = s2
    sa = np.zeros((8, 14), np.float32)
    saw = p["saw"][0]                                           # [2,7,7]
    for ch in range(2):
        for dy in range(7):
            for dx in range(7):
                sa += saw[ch, dy, dx] * pad[ch, dy : dy + 8, dx : dx + 14]
    v = (v * (1.0 / (1.0 + np.exp(-sa)))[None]).astype(np.float32)
    sd = v.reshape(256, 112)
    dcat = np.concatenate([p["Dict"], sd], axis=1).astype(np.float32)   # [256, 624]
    return ext, dcat


def _w8_blob(key, W1, W2, W3, n3, q1, q2, q3):
    """Per-layer fp8 weight blobs with pre-scales q1/q2/q3 applied."""
    if key in _WBLOB_CACHE:
        return _WBLOB_CACHE[key]
    a1 = np.ascontiguousarray(
        (W1 * q1).reshape(2, 128, 1024).transpose(1, 0, 2).reshape(128, 2048)).astype(F8NP)
    a2 = np.ascontiguousarray(
        (W2 * q2).reshape(8, 128, 512).transpose(1, 0, 2).reshape(128, 4096)).astype(F8NP)
    a3 = np.ascontiguousarray(
        (W3 * q3).reshape(4, 128, n3).transpose(1, 0, 2).reshape(128, 4 * n3)).astype(F8NP)
    _WBLOB_CACHE[key] = (a1, a2, a3)
    return _WBLOB_CACHE[key]


def _cols(v, ng):
    """[ng*128] (or shorter, zero-padded) -> [128, ng] column layout."""
    out = np.zeros(ng * 128, np.float32)
    out[: v.shape[0]] = v
    return out.reshape(ng, 128).T


def _wmax(W):
    return float(np.abs(W).max()) + 1e-30


def _mlp_quant(src, ext, W1, b1, W2, b2, W3, b3, n3, nb3, cdiv, centered):
    """Returns (w8 blob, cst [128,ncol]) for one MLP on one sample."""
    h1e = np.maximum(ext @ W1 + b1, 0.0)
    h2e = np.maximum(h1e @ W2 + b2, 0.0)
    if centered:
        mu1 = h1e.mean(0).astype(np.float32)
        mu2 = h2e.mean(0).astype(np.float32)
    else:
        mu1 = np.zeros_like(b1)
        mu2 = np.zeros_like(b2)
    a1 = float(np.abs(h1e - mu1).max())
    a2 = float(np.abs(h2e - mu2).max())
    s1 = min(_p2(240.0 / (16.0 * (a1 + 1e-6))), 2.0 ** 16)
    s2 = min(_p2(240.0 / (16.0 * (a2 + 1e-6))), 2.0 ** 16)
    q3 = _p2(240.0 / (2.0 * _wmax(W3)))
    if centered:
        # h8 = (relu(psum + a*b) - a*mu) * C2IMM on the DVE; alpha = SU*q1 etc.
        q1 = min(s1 / (SU * C2IMM), _p2(240.0 / (2.0 * _wmax(W1))))
        q2 = min(s2 / (s1 * C2IMM), _p2(240.0 / (2.0 * _wmax(W2))))
        al1 = SU * q1
        al2 = s1 * q2
        kvec = [0.0, 0.0, 1.0 / (s1 * q2 * C2IMM * q3 * cdiv)]
        b1s = al1 * (b1 + 0.5 * W1.sum(0))
        mu1s = al1 * mu1
        b2s = al2 * (b2 + mu1 @ W2)
        mu2s = al2 * mu2
    else:
        # h8 = relu(psum*k + s*b) via Scalar ACT; free choice of q1/q2
        q1 = _p2(240.0 / (2.0 * _wmax(W1)))
        q2 = _p2(240.0 / (2.0 * _wmax(W2)))
        kvec = [s1 / (SU * q1), s2 / (s1 * q2), 1.0 / (s2 * q3 * cdiv)]
        b1s = s1 * (b1 + 0.5 * W1.sum(0))
        mu1s = np.zeros_like(b1)
        b2s = s2 * b2
        mu2s = np.zeros_like(b2)
    b3s = ((b3 + mu2 @ W3) / cdiv).astype(np.float32)
    blob = _w8_blob((src, q1, q2, q3), W1, W2, W3, n3, q1, q2, q3)
    ncol = 24 + nb3 + 3
    cst = np.zeros((128, ncol), np.float32)
    cst[:, CST_B1 : CST_B1 + 8] = _cols(b1s.astype(np.float32), 8)
    cst[:, CST_MU1 : CST_MU1 + 8] = _cols(mu1s.astype(np.float32), 8)
    cst[:, CST_B2 : CST_B2 + 4] = _cols(b2s.astype(np.float32), 4)
    cst[:, CST_MU2 : CST_MU2 + 4] = _cols(mu2s.astype(np.float32), 4)
    cst[:, CST_B3 : CST_B3 + nb3] = _cols(b3s, nb3)
    cst[:, 24 + nb3] = kvec[0]
    cst[:, 24 + nb3 + 1] = kvec[1]
    cst[:, 24 + nb3 + 2] = kvec[2]
    return blob, cst


def _fold(pfull):
    # pfull [256, 12769] feature-major -> overlap-add [128,128]
    out = np.zeros((128, 128), np.float32)
    pr = pfull.reshape(16, 16, PR, PR)
    for kh in range(16):
        for kw in range(16):
            out[kh : kh + PR, kw : kw + PR] += pr[kh, kw]
    return out


def _assemble(chunks):
    # chunks: list of 4 arrays [256, 3193] -> [256, 12769]
    full = np.empty((256, L_FULL), np.float32)
    for q in range(4):
        full[:, T0S[q] : T0S[q] + LC] = chunks[q]
    return full


def _unfold_full(img):
    """[256, 12769] feature-major unfold of one [128,128] image."""
    sw = np.lib.stride_tricks.sliding_window_view(img, (16, 16))
    return sw.transpose(2, 3, 0, 1).reshape(256, L_FULL)


def _build_ufall(uf_full, t0):
    """Per-core pre-unfolded uf blob [128, sum(2*N_t)] from token t0."""
    cols = []
    ofs = 0
    for Nv in TILE_NV:
        N = Nv + (Nv % 2)
        blk = uf_full[:, t0 + ofs : t0 + ofs + Nv]
        if N > Nv:
            blk = np.concatenate([blk, np.full((256, N - Nv), 0.5, np.float32)], axis=1)
        cols.append(blk.reshape(2, 128, N).transpose(1, 0, 2).reshape(128, 2 * N))
        ofs += Nv
    return np.ascontiguousarray(np.concatenate(cols, axis=1), dtype=np.float32)


# --------------------------------------------------------------------------
# stage driver
# --------------------------------------------------------------------------

def _run_stage(nc, imgs, p, lam_pre, pd_pre, c_val, results_holder=None, trace=False, tmpdir=None):
    eye_b = np.eye(128, dtype=np.float32)
    per_sample = []
    uf_fulls = []
    for n in range(2):
        uf_fulls.append(_unfold_full(imgs[n]))
        ext, dcat = _host_sd(imgs[n], p, c_val)
        quants = {}
        for dev_pre, src, cdiv in (("pd", pd_pre, c_val), ("lam", lam_pre, c_val), ("w", "w", 1.0)):
            quants[dev_pre] = _mlp_quant(
                src, ext, p[src + "1w"], p[src + "1b"], p[src + "2w"], p[src + "2b"],
                p[src + "3w"], p[src + "3b"], N3[dev_pre], NB3[dev_pre], cdiv,
                centered=(dev_pre != "w"),
            )
        dcat_b = dcat.reshape(2, 128, DD).transpose(1, 0, 2).reshape(128, 2 * DD)
        dT = np.zeros((640, 256), np.float32)
        dT[:DD] = dcat.T
        dcatT_b = dT.reshape(5, 128, 256).transpose(1, 0, 2).reshape(128, 5 * 256)
        per_sample.append((quants, np.ascontiguousarray(dcat_b),
                           np.ascontiguousarray(dcatT_b)))

    in_maps = []
    for core in range(8):
        n, q = divmod(core, 4)
        quants, dcat_b, dcatT_b = per_sample[n]
        m = {}
        m["ufall"] = _build_ufall(uf_fulls[n], T0S[q])
        for pre in ("pd", "lam", "w"):
            for li in (1, 2, 3):
                m[f"{pre}w8_{li}"] = quants[pre][0][li - 1]
            m[f"{pre}cst"] = quants[pre][1]
        m["dcat_b"] = dcat_b
        m["dcatT_b"] = dcatT_b
        m["eye_b"] = eye_b
        in_maps.append(m)

    import time as _time
    last = None
    for _attempt in range(4):
        try:
            res = run_bass_kernel_spmd(nc, in_maps, list(range(8)), trace=trace, tmpdir=tmpdir)
            break
        except Exception as e:  # transient NRT device errors: retry after backoff
            last = e
            _time.sleep(5.0 + 10.0 * _attempt)
    else:
        raise last
    if results_holder is not None:
        results_holder.append(res)

    out = np.empty((2, 128, 128), np.float32)
    for n in range(2):
        px = _assemble([np.asarray(res.results[4 * n + q]["px_o"]).astype(np.float32)
                        for q in range(4)])
        wgf = _assemble([np.asarray(res.results[4 * n + q]["wg_o"]).astype(np.float32)
                        for q in range(4)])
        num = _fold(px)
        den = _fold(wgf)
        out[n] = num / den
    return out


def kernel(**inputs) -> np.ndarray:
    p = {k: np.asarray(v, np.float32) for k, v in inputs.items()}
    c_val = float(np.asarray(inputs["c"]))
    key = ("nc", c_val)
    if key not in _NC_CACHE:
        _NC_CACHE[key] = _build(c_val)
    nc = _NC_CACHE[key]
    x = p["x"]  # [2,1,128,128]
    imgs1 = [x[n, 0] for n in range(2)]
    res1 = _run_stage(nc, imgs1, p, "a", "p", c_val)
    imgs2 = [res1[n] for n in range(2)]
    res2 = _run_stage(nc, imgs2, p, "b", "q", c_val)
    return res2.reshape(2, 1, 128, 128).astype(np.float32)


# revision 31
# speedup vs baseline: 1.0882x; 1.0882x over previous
"""Trainium2 Bass kernel for nn_DenoisingNet_MLP_3 (LISTA denoiser, 2 stages).

Strategy: 8 cores = 2 samples x 4 patch-row chunks. The device runs the heavy
per-token pipeline (thr/Wg MLPs, y, 5 LISTA iterations, x_pred). The three
per-token MLPs run in fp8 (e4m3 DoubleRow matmuls, 2x PE rate) with host-side
calibration: inputs centered (uf-0.5, h-mean from the 112 ext tokens), per
tensor pow2 scales folded into activation scale/bias APs. The LISTA loop and
dictionary products stay fp32r. Elementwise work is spread across Scalar
(relu/copies), Vector (soft-threshold DVE, PSUM adds) and Pool (SBUF adds,
fp8 casts). The host runs the tiny per-sample ops (sd-MLP/CBAM -> Dcat),
unfold slicing and the overlap-add fold. One compiled NEFF serves both stages.
"""
import numpy as np
import ml_dtypes
import concourse.bass as bass
import concourse.bacc as bacc
import concourse.mybir as mybir
import concourse.tile as tile
from concourse.bass_utils import run_bass_kernel_spmd

fp32 = mybir.dt.float32
fp32r = mybir.dt.float32r
bf16 = mybir.dt.bfloat16
f8e4 = mybir.dt.float8e4
Alu = mybir.AluOpType
Act = mybir.ActivationFunctionType
DR = mybir.MatmulPerfMode.DoubleRow
F8NP = ml_dtypes.float8_e4m3
BF16NP = ml_dtypes.bfloat16

# ---- fused soft-threshold custom DVE op: out = sign(x) * relu(|x| - l) ----
import concourse.dve_ops as _dvo
from concourse.dve_spec import Spec as _Spec, Src0 as _S0, Src1 as _S1, Zero as _Z0, \
    C0 as _C0, C1 as _C1, C2 as _C2, relu as _relu, maxx as _maxx, lower as _lower
from concourse.dve_uop import DveOpSpec as _DveOpSpec


def _soft_ref(in0, in1, s0, s1, imm2):
    x = in0.astype(np.float32)
    return np.sign(x) * np.maximum(np.abs(x) - in1.astype(np.float32), 0.0)


def _rc_ref(in0, in1, s0, s1, imm2):
    x = in0.astype(np.float32)
    return (np.maximum(x + s0, 0.0) - s1) * imm2


def _reg_op(name, spec, rd1):
    op = _dvo.DveOp(name, spec, subdim=False, uops_sha={})
    if name not in _dvo._SUB_OPCODE_FOR_NAME:
        _dvo.OPS.append(op)
        _dvo._SUB_OPCODE_FOR_NAME[name] = _dvo._CUSTOM_DVE_ROW_BASE + len(_dvo.OPS) - 1
    for _ver in ("v3", "v4"):
        try:
            _sp = _DveOpSpec(name=name, opcode=_dvo.get_dve_sub_opcode(name),
                             uops=_lower(spec, ver=_ver), rd1_en=rd1)
            op.uops_sha[_ver] = _sp.sha(_ver)
        except Exception:
            pass
    return op


_SOFT_SPEC = _Spec(body=((_S0 > _Z0) - (_S0 < _Z0)) * _relu(_maxx(_S0, _Z0 - _S0) - _S1),
                   reference=_soft_ref)
SOFT_SHRINK_ANT = _reg_op("SOFT_SHRINK_ANT", _SOFT_SPEC, True)
_RC_SPEC = _Spec(body=(_relu(_S0 + _C0) - _C1) * _C2, reference=_rc_ref)
RELU_CENTER_ANT = _reg_op("RELU_CENTER_ANT", _RC_SPEC, False)

KP = 16            # patch size
P2 = 256           # patch features
DD = 624
PR = 113           # stride-1 patch grid is 113x113
L_FULL = PR * PR               # 12769 tokens per sample
T0S = [0, 3192, 6384, 9576]    # first token per core chunk (1-token overlaps)
LC = 3193                      # tokens per core
TILE_NV = [457, 456, 456, 456, 456, 456, 456]   # 7 token tiles per core
D_SZ = [128, 128, 128, 128, 112]
T_LISTA = 5
SU = 8.0                       # uf fp8 pre-scale: (uf - 0.5) * SU
C2IMM = 2.0 ** -6              # fixed post-scale of the fused relu-center DVE op
N3 = {"pd": 512, "lam": 112, "w": 256}
NB3 = {"pd": 4, "lam": 1, "w": 2}
# cst blob column offsets: b1s[8] mu1s[8] b2s[4] mu2s[4] b3s[nb3] k1 k2 k3
CST_B1, CST_MU1, CST_B2, CST_MU2, CST_B3 = 0, 8, 16, 20, 24
CST_K = {p: 24 + NB3[p] for p in N3}
CST_NCOL = {p: 24 + NB3[p] + 3 for p in N3}
W8_F = {p: 2048 + 4096 + 4 * N3[p] for p in N3}

_NC_CACHE: dict = {}
_WBLOB_CACHE: dict = {}


def _p2(x):
    return float(2.0 ** np.floor(np.log2(x)))


# --------------------------------------------------------------------------
# device program
# --------------------------------------------------------------------------

def _build(c_val: float):
    nc = bacc.Bacc("TRN2", target_bir_lowering=False, debug=False, num_devices=8)

    FT = sum(2 * (nv + nv % 2) for nv in TILE_NV)
    ufall_d = nc.dram_tensor("ufall", [128, FT], fp32r, kind="ExternalInput")
    w8d, cstd = {}, {}
    for pre in ("pd", "lam", "w"):
        for li, fl in ((1, 2048), (2, 4096), (3, 4 * N3[pre])):
            w8d[(pre, li)] = nc.dram_tensor(f"{pre}w8_{li}", [128, fl], f8e4, kind="ExternalInput")
        cstd[pre] = nc.dram_tensor(f"{pre}cst", [128, CST_NCOL[pre]], fp32, kind="ExternalInput")
    dcat_d = nc.dram_tensor("dcat_b", [128, 2 * DD], fp32r, kind="ExternalInput")
    dcatT_d = nc.dram_tensor("dcatT_b", [128, 5 * 256], fp32r, kind="ExternalInput")
    eye_d = nc.dram_tensor("eye_b", [128, 128], fp32r, kind="ExternalInput")
    px_o = nc.dram_tensor("px_o", [256, LC], bf16, kind="ExternalOutput")
    wg_o = nc.dram_tensor("wg_o", [256, LC], bf16, kind="ExternalOutput")

    inv_c = float(1.0 / c_val)

    with tile.TileContext(nc) as tc:
        with (
            tc.tile_pool(name="fx", bufs=1) as fx,
            tc.tile_pool(name="wk", bufs=1) as wk,
            tc.tile_pool(name="pp", bufs=1, space="PSUM") as pp,
        ):
            # ---- persistent loads: csts first (tiny, scalar queue); weight
            # layers stream on the gpsimd queue in exact first-use order so
            # compute starts as soon as the first 256KB lands ----
            ws, cs = {}, {}
            for pre in ("pd", "lam", "w"):
                ct = fx.tile([128, CST_NCOL[pre]], fp32, name=f"sb_{pre}cst")
                nc.scalar.dma_start(
                    out=ct[:],
                    in_=bass.AP(cstd[pre], 0, [[CST_NCOL[pre], 128], [1, CST_NCOL[pre]]]),
                )
                cs[pre] = ct
            for pre in ("pd", "lam", "w"):
                for li, fl in ((1, 2048), (2, 4096), (3, 4 * N3[pre])):
                    t = fx.tile([128, fl], f8e4, name=f"sb_{pre}w8_{li}")
                    nc.gpsimd.dma_start(
                        out=t[:], in_=bass.AP(w8d[(pre, li)], 0, [[fl, 128], [1, fl]])
                    )
                    ws[(pre, li)] = t
            dcat = fx.tile([128, 2 * DD], fp32r, name="sb_dcat")
            nc.gpsimd.dma_start(
                out=dcat[:], in_=bass.AP(dcat_d, 0, [[2 * DD, 128], [1, 2 * DD]])
            )
            dcatT = fx.tile([128, 5 * 256], fp32r, name="sb_dcatT")
            nc.gpsimd.dma_start(
                out=dcatT[:], in_=bass.AP(dcatT_d, 0, [[5 * 256, 128], [1, 5 * 256]])
            )
            eye = fx.tile([128, 128], fp32r, name="sb_eye")
            nc.gpsimd.dma_start(
                out=eye[:], in_=bass.AP(eye_d, 0, [[128, 128], [1, 128]])
            )
            # dcatN = -dcat/c derived on-device instead of a 640KB upload
            dcatN = fx.tile([128, 2 * DD], fp32r, name="sb_dcatN")
            nc.scalar.mul(dcatN[:], dcat[:], -inv_c)

            # ---- per-tile geometry ----
            def geom(t):
                Nv = TILE_NV[t]
                return None, None, Nv, Nv + (Nv % 2), sum(TILE_NV[:t])

            def ufbase(t):
                return sum(2 * (nv + nv % 2) for nv in TILE_NV[:t])

            st = [dict() for _ in TILE_NV]   # per-tile live handles

            # ---- prologue pieces (overlappable) ----
            def p_unfold(t):
                _, _, Nv, N, tok0 = geom(t)
                uf = wk.tile([128, 2 * N], fp32r, name=f"uf{t}", tag="uf", bufs=2)
                uf8 = wk.tile([128, 2 * N], f8e4, name=f"uf8_{t}", tag="uf8", bufs=2)
                nc.sync.dma_start(
                    out=uf[:],
                    in_=bass.AP(ufall_d, ufbase(t), [[FT, 128], [1, 2 * N]]),
                )
                # uf8 = fp8((uf - 0.5) * SU), one cheap DVE pass
                nc.vector.tensor_scalar(
                    uf8[:], uf[:].bitcast(fp32), 0.5, SU, Alu.subtract, Alu.mult
                )
                st[t]["uf"] = uf
                st[t]["uf8"] = uf8
                st[t]["thr"] = wk.tile([128, 5 * N], fp32, name=f"thr{t}", tag="thr", bufs=2)
                st[t]["wg"] = wk.tile([128, 2 * N], bf16, name=f"wg{t}", tag="wg", bufs=2)

            def p_l1(t, pre):
                _, _, _, N, _ = geom(t)
                uf8 = st[t]["uf8"][:].rearrange("p (k n) -> p k n", k=2)
                ct = cs[pre]
                h18 = wk.tile([128, 8 * N], f8e4, name=f"h18_{t}_{pre}", tag="h18", bufs=2)
                w1v = ws[(pre, 1)][:].rearrange("p (j m) -> p j m", j=2)
                for g in range(8):
                    ps1 = pp.tile([128, N], fp32, name=f"ps1_{t}_{pre}_{g}", tag="ps", bufs=8)
                    nc.tensor.matmul(
                        ps1[:], w1v[:, :, g * 128 : g * 128 + 128], uf8,
                        start=True, stop=True, perf_mode=DR,
                    )
                    if pre == "w":
                        # uncentered: h18 = relu(psum*k1 + s1*b1c) directly (Scalar)
                        nc.scalar.activation(
                            h18[:, g * N : (g + 1) * N], ps1[:], Act.Relu,
                            bias=ct[:, CST_B1 + g : CST_B1 + g + 1],
                            scale=ct[:, CST_K[pre] : CST_K[pre] + 1],
                        )
                    else:
                        # fused relu+center+cast on Vector: (relu(ps+a1*b1c)-a1*mu1)*C2
                        nc.vector._custom_dve(
                            RELU_CENTER_ANT, out=h18[:, g * N : (g + 1) * N],
                            in0=ps1[:],
                            s0=ct[:, CST_B1 + g : CST_B1 + g + 1],
                            s1=ct[:, CST_MU1 + g : CST_MU1 + g + 1],
                            imm2=C2IMM,
                        )
                st[t][f"h18_{pre}"] = h18

            def p_l2(t, pre):
                _, _, _, N, _ = geom(t)
                h18 = st[t][f"h18_{pre}"][:].rearrange("p (k n) -> p k n", k=8)
                ct = cs[pre]
                h28 = wk.tile([128, 4 * N], f8e4, name=f"h28_{t}_{pre}", tag="h28", bufs=2)
                w2v = ws[(pre, 2)][:].rearrange("p (j2 j m) -> p j2 j m", j2=4, j=2)
                for g in range(4):
                    ps2 = pp.tile([128, N], fp32, name=f"ps2_{t}_{pre}_{g}", tag="ps", bufs=8)
                    for j2 in range(4):
                        nc.tensor.matmul(
                            ps2[:], w2v[:, j2, :, g * 128 : g * 128 + 128],
                            h18[:, 2 * j2 : 2 * j2 + 2, :],
                            start=(j2 == 0), stop=(j2 == 3), perf_mode=DR,
                        )
                    if pre == "w":
                        nc.scalar.activation(
                            h28[:, g * N : (g + 1) * N], ps2[:], Act.Relu,
                            bias=ct[:, CST_B2 + g : CST_B2 + g + 1],
                            scale=ct[:, CST_K[pre] + 1 : CST_K[pre] + 2],
                        )
                    else:
                        nc.vector._custom_dve(
                            RELU_CENTER_ANT, out=h28[:, g * N : (g + 1) * N],
                            in0=ps2[:],
                            s0=ct[:, CST_B2 + g : CST_B2 + g + 1],
                            s1=ct[:, CST_MU2 + g : CST_MU2 + g + 1],
                            imm2=C2IMM,
                        )
                st[t][f"h28_{pre}"] = h28

            def p_l3(t, pre):
                _, _, _, N, _ = geom(t)
                n3 = N3[pre]
                h28 = st[t][f"h28_{pre}"][:].rearrange("p (k n) -> p k n", k=4)
                ct = cs[pre]
                thr, wg = st[t]["thr"], st[t]["wg"]
                w3v = ws[(pre, 3)][:].rearrange("p (j2 j m) -> p j2 j m", j2=2, j=2)
                for mo in range(NB3[pre]):
                    sz = min(128, n3 - mo * 128)
                    ps3 = pp.tile([128, N], fp32, name=f"ps3_{t}_{pre}_{mo}", tag="ps", bufs=8)
                    for j2 in range(2):
                        nc.tensor.matmul(
                            ps3[0:sz], w3v[:, j2, :, mo * 128 : mo * 128 + sz],
                            h28[:, 2 * j2 : 2 * j2 + 2, :],
                            start=(j2 == 0), stop=(j2 == 1), perf_mode=DR,
                        )
                    kap = ct[:, CST_K[pre] + 2 : CST_K[pre] + 3]
                    if pre == "pd":
                        nc.scalar.activation(
                            thr[:, mo * N : (mo + 1) * N], ps3[:], Act.Identity,
                            bias=ct[:, CST_B3 + mo : CST_B3 + mo + 1], scale=kap,
                        )
                    elif pre == "lam":
                        nc.scalar.activation(
                            thr[0:112, 4 * N : 5 * N], ps3[0:112], Act.Identity,
                            bias=ct[0:112, CST_B3 : CST_B3 + 1], scale=kap[0:112],
                        )
                    else:
                        nc.scalar.activation(
                            wg[:, mo * N : (mo + 1) * N], ps3[:], Act.Sigmoid,
                            bias=ct[:, CST_B3 + mo : CST_B3 + mo + 1], scale=kap,
                        )

            def p_y(t):
                _, _, _, N, _ = geom(t)
                uf, thr = st[t]["uf"], st[t]["thr"]
                yc = wk.tile([128, 5 * N], fp32r, name=f"yc{t}", tag="yc", bufs=2)
                z = wk.tile([128, 5 * N], fp32r, name=f"z{t}_0", tag="z", bufs=2)
                for mc in range(5):
                    sz = D_SZ[mc]
                    d0 = 128 * mc
                    psy = pp.tile([128, N], fp32, name=f"psy_{t}_{mc}", tag="ps", bufs=8)
                    nc.tensor.matmul(
                        psy[0:sz], dcat[:, d0 : d0 + sz], uf[:, 0:N], start=True, stop=False
                    )
                    nc.tensor.matmul(
                        psy[0:sz], dcat[:, DD + d0 : DD + d0 + sz], uf[:, N : 2 * N],
                        start=False, stop=True,
                    )
                    nc.scalar.mul(yc[0:sz, mc * N : (mc + 1) * N], psy[0:sz], inv_c)
                    nc.vector._custom_dve(
                        SOFT_SHRINK_ANT,
                        out=z[0:sz, mc * N : (mc + 1) * N],
                        in0=psy[0:sz], in1=thr[0:sz, mc * N : (mc + 1) * N],
                    )
                st[t]["yc"] = yc
                st[t]["z"] = z

            def prologue_pieces(t):
                yield from (
                    lambda: p_unfold(t),
                    lambda: p_l1(t, "pd"), lambda: p_l2(t, "pd"), lambda: p_l3(t, "pd"),
                    lambda: p_l1(t, "lam"), lambda: p_l2(t, "lam"), lambda: p_l3(t, "lam"),
                    lambda: p_l1(t, "w"), lambda: p_l2(t, "w"), lambda: p_l3(t, "w"),
                    lambda: p_y(t),
                )

            def lista_iter_a(t, it):
                _, _, _, N, _ = geom(t)
                yc, z = st[t]["yc"], st[t]["z"]
                # psl_mc = yc + z - DcatT@G/c; identity matmuls first (operands
                # ready at iter start), psg next, DcatN accumulation in iter_b
                psl = []
                for mc in range(5):
                    sz = D_SZ[mc]
                    ps = pp.tile([128, N], fp32, name=f"psl_{t}_{it}_{mc}", tag="ps", bufs=8)
                    psl.append(ps)
                    nc.tensor.matmul(
                        ps[0:sz], eye[0:sz, 0:sz], yc[0:sz, mc * N : (mc + 1) * N],
                        start=True, stop=False,
                    )
                    nc.tensor.matmul(
                        ps[0:sz], eye[0:sz, 0:sz], z[0:sz, mc * N : (mc + 1) * N],
                        start=False, stop=False,
                    )
                # G = Dcat @ z   [256, N]
                g = wk.tile([128, 2 * N], fp32r, name=f"g{t}_{it}", tag="g", bufs=2)
                for fc in range(2):
                    psg = pp.tile([128, N], fp32, name=f"psg_{t}_{it}_{fc}", tag="ps", bufs=8)
                    for kc in range(5):
                        szk = D_SZ[kc]
                        nc.tensor.matmul(
                            psg[:],
                            dcatT[0:szk, kc * 256 + fc * 128 : kc * 256 + fc * 128 + 128],
                            z[0:szk, kc * N : (kc + 1) * N],
                            start=(kc == 0), stop=(kc == 4),
                        )
                    nc.scalar.copy(g[:, fc * N : (fc + 1) * N], psg[:])
                st[t]["psl"] = psl
                st[t]["g"] = g

            def lista_iter_b(t, it):
                _, _, _, N, _ = geom(t)
                thr = st[t]["thr"]
                psl, g = st[t]["psl"], st[t]["g"]
                zn = wk.tile([128, 5 * N], fp32r, name=f"z{t}_{it + 1}", tag="z", bufs=2)
                for mc in range(5):
                    sz = D_SZ[mc]
                    d0 = 128 * mc
                    ps = psl[mc]
                    for fc in range(2):
                        nc.tensor.matmul(
                            ps[0:sz],
                            dcatN[:, fc * DD + d0 : fc * DD + d0 + sz],
                            g[:, fc * N : (fc + 1) * N],
                            start=False, stop=(fc == 1),
                        )
                    nc.vector._custom_dve(
                        SOFT_SHRINK_ANT,
                        out=zn[0:sz, mc * N : (mc + 1) * N],
                        in0=ps[0:sz], in1=thr[0:sz, mc * N : (mc + 1) * N],
                    )
                st[t]["z"] = zn

            def emit_xp(t):
                _, _, Nv, N, tok0 = geom(t)
                z, wg = st[t]["z"], st[t]["wg"]
                for fc in range(2):
                    psx = pp.tile([128, N], fp32, name=f"psx_{t}_{fc}", tag="ps", bufs=8)
                    for kc in range(5):
                        szk = D_SZ[kc]
                        nc.tensor.matmul(
                            psx[:],
                            dcatT[0:szk, kc * 256 + fc * 128 : kc * 256 + fc * 128 + 128],
                            z[0:szk, kc * N : (kc + 1) * N],
                            start=(kc == 0), stop=(kc == 4),
                        )
                    nc.vector.tensor_scalar(psx[:], psx[:], 0.0, 1.0, Alu.max, Alu.min)
                    px = wk.tile([128, N], bf16, name=f"px{t}_{fc}", tag="px", bufs=2)
                    nc.vector.tensor_tensor(px[:], psx[:], wg[:, fc * N : (fc + 1) * N], Alu.mult)
                    nc.scalar.dma_start(
                        out=bass.AP(px_o, fc * 128 * LC + tok0, [[LC, 128], [1, Nv]]),
                        in_=px[:, 0:Nv],
                    )
                    nc.scalar.dma_start(
                        out=bass.AP(wg_o, fc * 128 * LC + tok0, [[LC, 128], [1, Nv]]),
                        in_=wg[:, fc * N : fc * N + Nv],
                    )

            # ---- driver: software-pipeline tiles; next-tile prologue pieces
            # are slotted at BOTH half-iteration boundaries so the PE always
            # has independent work while g-copies / softs drain ----
            n_tiles = len(TILE_NV)
            for piece in prologue_pieces(0):
                piece()
            n_slots = 2 * T_LISTA
            for t in range(n_tiles):
                nxt = list(prologue_pieces(t + 1)) if t + 1 < n_tiles else []
                slots = [nxt[(len(nxt) * i) // n_slots : (len(nxt) * (i + 1)) // n_slots]
                         for i in range(n_slots)]
                for it in range(T_LISTA):
                    lista_iter_a(t, it)
                    for piece in slots[2 * it]:
                        piece()
                    lista_iter_b(t, it)
                    for piece in slots[2 * it + 1]:
                        piece()
                emit_xp(t)

    nc.compile()
    return nc


# --------------------------------------------------------------------------
# host-side small ops (per sample): ext -> sd MLP -> CBAM -> Dcat
# --------------------------------------------------------------------------

def _host_sd(img2d, p, c_val):
    # ext: stride-8 unfold, every 2nd patch, first 112   [112, 256]
    ext = np.empty((112, 256), np.float32)
    for tt in range(112):
        ir, ic = divmod(2 * tt, 15)
        ext[tt] = img2d[8 * ir : 8 * ir + 16, 8 * ic : 8 * ic + 16].reshape(256)
    h = ext
    for wname, bname in (("s1w", "s1b"), ("s2w", "s2b"), ("s3w", "s3b")):
        h = np.maximum(h @ p[wname] + p[bname], 0.0, dtype=np.float32)
    sd = (h @ p["s4w"] + p["s4b"]).astype(np.float32)          # [112, 256]
    nrm = np.maximum(np.linalg.norm(sd, axis=-1, keepdims=True), 1e-12)
    sd = (sd / nrm).astype(np.float32)
    v = sd.T.reshape(256, 8, 14)                                # channels, 8x14
    def camlp(vec):
        return np.maximum(vec @ p["caw1"], 0.0) @ p["caw2"]
    ca = 1.0 / (1.0 + np.exp(-(camlp(v.mean(axis=(1, 2))) + camlp(v.max(axis=(1, 2))))))
    v = (v * ca[:, None, None]).astype(np.float32)
    s2 = np.stack([v.mean(axis=0), v.max(axis=0)])              # [2, 8, 14]
    pad = np.zeros((2, 14, 20), np.float32)
    pad[:, 3:11, 3:17] = s2
    sa = np.zeros((8, 14), np.float32)
    saw = p["saw"][0]                                           # [2,7,7]
    for ch in range(2):
        for dy in range(7):
            for dx in range(7):
                sa += saw[ch, dy, dx] * pad[ch, dy : dy + 8, dx : dx + 14]
    v = (v * (1.0 / (1.0 + np.exp(-sa)))[None]).astype(np.float32)
    sd = v.reshape(256, 112)
    dcat = np.concatenate([p["Dict"], sd], axis=1).astype(np.float32)   # [256, 624]
    return ext, dcat


def _w8_blob(key, W1, W2, W3, n3, q1, q2, q3):
    """Per-layer fp8 weight blobs with pre-scales q1/q2/q3 applied."""
    if key in _WBLOB_CACHE:
        return _WBLOB_CACHE[key]
    a1 = np.ascontiguousarray(
        (W1 * q1).reshape(2, 128, 1024).transpose(1, 0, 2).reshape(128, 2048)).astype(F8NP)
    a2 = np.ascontiguousarray(
        (W2 * q2).reshape(8, 128, 512).transpose(1, 0, 2).reshape(128, 4096)).astype(F8NP)
    a3 = np.ascontiguousarray(
        (W3 * q3).reshape(4, 128, n3).transpose(1, 0, 2).reshape(128, 4 * n3)).astype(F8NP)
    _WBLOB_CACHE[key] = (a1, a2, a3)
    return _WBLOB_CACHE[key]


def _cols(v, ng):
    """[ng*128] (or shorter, zero-padded) -> [128, ng] column layout."""
    out = np.zeros(ng * 128, np.float32)
    out[: v.shape[0]] = v
    return out.reshape(ng, 128).T


def _wmax(W):
    return float(np.abs(W).max()) + 1e-30


def _mlp_quant(src, ext, W1, b1, W2, b2, W3, b3, n3, nb3, cdiv, centered):
    """Returns (w8 blob, cst [128,ncol]) for one MLP on one sample."""
    h1e = np.maximum(ext @ W1 + b1, 0.0)
    h2e = np.maximum(h1e @ W2 + b2, 0.0)
    if centered:
        mu1 = h1e.mean(0).astype(np.float32)
        mu2 = h2e.mean(0).astype(np.float32)
    else:
        mu1 = np.zeros_like(b1)
        mu2 = np.zeros_like(b2)
    a1 = float(np.abs(h1e - mu1).max())
    a2 = float(np.abs(h2e - mu2).max())
    s1 = min(_p2(240.0 / (16.0 * (a1 + 1e-6))), 2.0 ** 16)
    s2 = min(_p2(240.0 / (16.0 * (a2 + 1e-6))), 2.0 ** 16)
    q3 = _p2(240.0 / (2.0 * _wmax(W3)))
    if centered:
        # h8 = (relu(psum + a*b) - a*mu) * C2IMM on the DVE; alpha = SU*q1 etc.
        q1 = min(s1 / (SU * C2IMM), _p2(240.0 / (2.0 * _wmax(W1))))
        q2 = min(s2 / (s1 * C2IMM), _p2(240.0 / (2.0 * _wmax(W2))))
        al1 = SU * q1
        al2 = s1 * q2
        kvec = [0.0, 0.0, 1.0 / (s1 * q2 * C2IMM * q3 * cdiv)]
        b1s = al1 * (b1 + 0.5 * W1.sum(0))
        mu1s = al1 * mu1
        b2s = al2 * (b2 + mu1 @ W2)
        mu2s = al2 * mu2
    else:
        # h8 = relu(psum*k + s*b) via Scalar ACT; free choice of q1/q2
        q1 = _p2(240.0 / (2.0 * _wmax(W1)))
        q2 = _p2(240.0 / (2.0 * _wmax(W2)))
        kvec = [s1 / (SU * q1), s2 / (s1 * q2), 1.0 / (s2 * q3 * cdiv)]
        b1s = s1 * (b1 + 0.5 * W1.sum(0))
        mu1s = np.zeros_like(b1)
        b2s = s2 * b2
        mu2s = np.zeros_like(b2)
    b3s = ((b3 + mu2 @ W3) / cdiv).astype(np.float32)
    blob = _w8_blob((src, q1, q2, q3), W1, W2, W3, n3, q1, q2, q3)
    ncol = 24 + nb3 + 3
    cst = np.zeros((128, ncol), np.float32)
    cst[:, CST_B1 : CST_B1 + 8] = _cols(b1s.astype(np.float32), 8)
    cst[:, CST_MU1 : CST_MU1 + 8] = _cols(mu1s.astype(np.float32), 8)
    cst[:, CST_B2 : CST_B2 + 4] = _cols(b2s.astype(np.float32), 4)
    cst[:, CST_MU2 : CST_MU2 + 4] = _cols(mu2s.astype(np.float32), 4)
    cst[:, CST_B3 : CST_B3 + nb3] = _cols(b3s, nb3)
    cst[:, 24 + nb3] = kvec[0]
    cst[:, 24 + nb3 + 1] = kvec[1]
    cst[:, 24 + nb3 + 2] = kvec[2]
    return blob, cst


def _fold(pfull):
    # pfull [256, 12769] feature-major -> overlap-add [128,128]
    out = np.zeros((128, 128), np.float32)
    pr = pfull.reshape(16, 16, PR, PR)
    for kh in range(16):
        for kw in range(16):
            out[kh : kh + PR, kw : kw + PR] += pr[kh, kw]
    return out


def _assemble(chunks):
    # chunks: list of 4 arrays [256, 3193] -> [256, 12769]
    full = np.empty((256, L_FULL), np.float32)
    for q in range(4):
        full[:, T0S[q] : T0S[q] + LC] = chunks[q]
    return full


def _unfold_full(img):
    """[256, 12769] feature-major unfold of one [128,128] image."""
    sw = np.lib.stride_tricks.sliding_window_view(img, (16, 16))
    return sw.transpose(2, 3, 0, 1).reshape(256, L_FULL)


def _build_ufall(uf_full, t0):
    """Per-core pre-unfolded uf blob [128, sum(2*N_t)] from token t0."""
    cols = []
    ofs = 0
    for Nv in TILE_NV:
        N = Nv + (Nv % 2)
        blk = uf_full[:, t0 + ofs : t0 + ofs + Nv]
        if N > Nv:
            blk = np.concatenate([blk, np.full((256, N - Nv), 0.5, np.float32)], axis=1)
        cols.append(blk.reshape(2, 128, N).transpose(1, 0, 2).reshape(128, 2 * N))
        ofs += Nv
    return np.ascontiguousarray(np.concatenate(cols, axis=1), dtype=np.float32)


# --------------------------------------------------------------------------
# stage driver
# --------------------------------------------------------------------------

def _run_stage(nc, imgs, p, lam_pre, pd_pre, c_val, results_holder=None, trace=False, tmpdir=None):
    eye_b = np.eye(128, dtype=np.float32)
    per_sample = []
    uf_fulls = []
    for n in range(2):
        uf_fulls.append(_unfold_full(imgs[n]))
        ext, dcat = _host_sd(imgs[n], p, c_val)
        quants = {}
        for dev_pre, src, cdiv in (("pd", pd_pre, c_val), ("lam", lam_pre, c_val), ("w", "w", 1.0)):
            quants[dev_pre] = _mlp_quant(
                src, ext, p[src + "1w"], p[src + "1b"], p[src + "2w"], p[src + "2b"],
                p[src + "3w"], p[src + "3b"], N3[dev_pre], NB3[dev_pre], cdiv,
                centered=(dev_pre != "w"),
            )
        dcat_b = dcat.reshape(2, 128, DD).transpose(1, 0, 2).reshape(128, 2 * DD)
        dT = np.zeros((640, 256), np.float32)
        dT[:DD] = dcat.T
        dcatT_b = dT.reshape(5, 128, 256).transpose(1, 0, 2).reshape(128, 5 * 256)
        per_sample.append((quants, np.ascontiguousarray(dcat_b),
                           np.ascontiguousarray(dcatT_b)))

    in_maps = []
    for core in range(8):
        n, q = divmod(core, 4)
        quants, dcat_b, dcatT_b = per_sample[n]
        m = {}
        m["ufall"] = _build_ufall(uf_fulls[n], T0S[q])
        for pre in ("pd", "lam", "w"):
            for li in (1, 2, 3):
                m[f"{pre}w8_{li}"] = quants[pre][0][li - 1]
            m[f"{pre}cst"] = quants[pre][1]
        m["dcat_b"] = dcat_b
        m["dcatT_b"] = dcatT_b
        m["eye_b"] = eye_b
        in_maps.append(m)

    import time as _time
    last = None
    for _attempt in range(4):
        try:
            res = run_bass_kernel_spmd(nc, in_maps, list(range(8)), trace=trace, tmpdir=tmpdir)
            break
        except Exception as e:  # transient NRT device errors: retry after backoff
            last = e
            _time.sleep(5.0 + 10.0 * _attempt)
    else:
        raise last
    if results_holder is not None:
        results_holder.append(res)

    out = np.empty((2, 128, 128), np.float32)
    for n in range(2):
        px = _assemble([np.asarray(res.results[4 * n + q]["px_o"]).astype(np.float32)
                        for q in range(4)])
        wgf = _assemble([np.asarray(res.results[4 * n + q]["wg_o"]).astype(np.float32)
                        for q in range(4)])
        num = _fold(px)
        den = _fold(wgf)
        out[n] = num / den
    return out


def kernel(**inputs) -> np.ndarray:
    p = {k: np.asarray(v, np.float32) for k, v in inputs.items()}
    c_val = float(np.asarray(inputs["c"]))
    key = ("nc", c_val)
    if key not in _NC_CACHE:
        _NC_CACHE[key] = _build(c_val)
    nc = _NC_CACHE[key]
    x = p["x"]  # [2,1,128,128]
    imgs1 = [x[n, 0] for n in range(2)]
    res1 = _run_stage(nc, imgs1, p, "a", "p", c_val)
    imgs2 = [res1[n] for n in range(2)]
    res2 = _run_stage(nc, imgs2, p, "b", "q", c_val)
    return res2.reshape(2, 1, 128, 128).astype(np.float32)


# revision 32
# speedup vs baseline: 1.1101x; 1.0201x over previous
"""Trainium2 Bass kernel for nn_DenoisingNet_MLP_3 (LISTA denoiser, 2 stages).

Strategy: 8 cores = 2 samples x 4 patch-row chunks. The device runs the heavy
per-token pipeline (thr/Wg MLPs, y, 5 LISTA iterations, x_pred). The three
per-token MLPs run in fp8 (e4m3 DoubleRow matmuls, 2x PE rate) with host-side
calibration: inputs centered (uf-0.5, h-mean from the 112 ext tokens), per
tensor pow2 scales folded into activation scale/bias APs. The LISTA loop and
dictionary products stay fp32r. Elementwise work is spread across Scalar
(relu/copies), Vector (soft-threshold DVE, PSUM adds) and Pool (SBUF adds,
fp8 casts). The host runs the tiny per-sample ops (sd-MLP/CBAM -> Dcat),
unfold slicing and the overlap-add fold. One compiled NEFF serves both stages.
"""
import numpy as np
import ml_dtypes
import concourse.bass as bass
import concourse.bacc as bacc
import concourse.mybir as mybir
import concourse.tile as tile
from concourse.bass_utils import run_bass_kernel_spmd

fp32 = mybir.dt.float32
fp32r = mybir.dt.float32r
bf16 = mybir.dt.bfloat16
f8e4 = mybir.dt.float8e4
Alu = mybir.AluOpType
Act = mybir.ActivationFunctionType
DR = mybir.MatmulPerfMode.DoubleRow
F8NP = ml_dtypes.float8_e4m3
BF16NP = ml_dtypes.bfloat16

# ---- fused soft-threshold custom DVE op: out = sign(x) * relu(|x| - l) ----
import concourse.dve_ops as _dvo
from concourse.dve_spec import Spec as _Spec, Src0 as _S0, Src1 as _S1, Zero as _Z0, \
    C0 as _C0, C1 as _C1, C2 as _C2, relu as _relu, maxx as _maxx, lower as _lower
from concourse.dve_uop import DveOpSpec as _DveOpSpec


def _soft_ref(in0, in1, s0, s1, imm2):
    x = in0.astype(np.float32)
    return np.sign(x) * np.maximum(np.abs(x) - in1.astype(np.float32), 0.0)


def _rc_ref(in0, in1, s0, s1, imm2):
    x = in0.astype(np.float32)
    return (np.maximum(x + s0, 0.0) - s1) * imm2


def _reg_op(name, spec, rd1):
    op = _dvo.DveOp(name, spec, subdim=False, uops_sha={})
    if name not in _dvo._SUB_OPCODE_FOR_NAME:
        _dvo.OPS.append(op)
        _dvo._SUB_OPCODE_FOR_NAME[name] = _dvo._CUSTOM_DVE_ROW_BASE + len(_dvo.OPS) - 1
    for _ver in ("v3", "v4"):
        try:
            _sp = _DveOpSpec(name=name, opcode=_dvo.get_dve_sub_opcode(name),
                             uops=_lower(spec, ver=_ver), rd1_en=rd1)
            op.uops_sha[_ver] = _sp.sha(_ver)
        except Exception:
            pass
    return op


_SOFT_SPEC = _Spec(body=((_S0 > _Z0) - (_S0 < _Z0)) * _relu(_maxx(_S0, _Z0 - _S0) - _S1),
                   reference=_soft_ref)
SOFT_SHRINK_ANT = _reg_op("SOFT_SHRINK_ANT", _SOFT_SPEC, True)
_RC_SPEC = _Spec(body=(_relu(_S0 + _C0) - _C1) * _C2, reference=_rc_ref)
RELU_CENTER_ANT = _reg_op("RELU_CENTER_ANT", _RC_SPEC, False)

KP = 16            # patch size
P2 = 256           # patch features
DD = 624
PR = 113           # stride-1 patch grid is 113x113
L_FULL = PR * PR               # 12769 tokens per sample
T0S = [0, 3192, 6384, 9576]    # first token per core chunk (1-token overlaps)
LC = 3193                      # tokens per core
TILE_NV = [457, 456, 456, 456, 456, 456, 456]   # 7 token tiles per core
D_SZ = [128, 128, 128, 128, 112]
T_LISTA = 5
SU = 8.0                       # uf fp8 pre-scale: (uf - 0.5) * SU
C2IMM = 2.0 ** -6              # fixed post-scale of the fused relu-center DVE op
N3 = {"pd": 512, "lam": 112, "w": 256}
NB3 = {"pd": 4, "lam": 1, "w": 2}
# cst blob column offsets: b1s[8] mu1s[8] b2s[4] mu2s[4] b3s[nb3] k1 k2 k3
CST_B1, CST_MU1, CST_B2, CST_MU2, CST_B3 = 0, 8, 16, 20, 24
CST_K = {p: 24 + NB3[p] for p in N3}
CST_NCOL = {p: 24 + NB3[p] + 3 for p in N3}
W8_F = {p: 2048 + 4096 + 4 * N3[p] for p in N3}

_NC_CACHE: dict = {}
_WBLOB_CACHE: dict = {}


def _p2(x):
    return float(2.0 ** np.floor(np.log2(x)))


# --------------------------------------------------------------------------
# device program
# --------------------------------------------------------------------------

def _build(c_val: float):
    nc = bacc.Bacc("TRN2", target_bir_lowering=False, debug=False, num_devices=8)

    FT = sum(2 * (nv + nv % 2) for nv in TILE_NV)
    ufall_d = nc.dram_tensor("ufall", [128, FT], fp32r, kind="ExternalInput")
    w8d, cstd = {}, {}
    for pre in ("pd", "lam", "w"):
        for li, fl in ((1, 2048), (2, 4096), (3, 4 * N3[pre])):
            w8d[(pre, li)] = nc.dram_tensor(f"{pre}w8_{li}", [128, fl], f8e4, kind="ExternalInput")
        cstd[pre] = nc.dram_tensor(f"{pre}cst", [128, CST_NCOL[pre]], fp32, kind="ExternalInput")
    dcat_d = nc.dram_tensor("dcat_b", [128, 2 * DD], fp32r, kind="ExternalInput")
    dcatT_d = nc.dram_tensor("dcatT_b", [128, 5 * 256], fp32r, kind="ExternalInput")
    eye_d = nc.dram_tensor("eye_b", [128, 128], fp32r, kind="ExternalInput")
    px_o = nc.dram_tensor("px_o", [256, LC], bf16, kind="ExternalOutput")
    wg_o = nc.dram_tensor("wg_o", [256, LC], bf16, kind="ExternalOutput")

    inv_c = float(1.0 / c_val)

    with tile.TileContext(nc) as tc:
        with (
            tc.tile_pool(name="fx", bufs=1) as fx,
            tc.tile_pool(name="wk", bufs=1) as wk,
            tc.tile_pool(name="pp", bufs=1, space="PSUM") as pp,
        ):
            # ---- persistent loads: csts first (tiny, scalar queue); weight
            # layers stream on the gpsimd queue in exact first-use order so
            # compute starts as soon as the first 256KB lands ----
            ws, cs = {}, {}
            for pre in ("pd", "lam", "w"):
                ct = fx.tile([128, CST_NCOL[pre]], fp32, name=f"sb_{pre}cst")
                nc.scalar.dma_start(
                    out=ct[:],
                    in_=bass.AP(cstd[pre], 0, [[CST_NCOL[pre], 128], [1, CST_NCOL[pre]]]),
                )
                cs[pre] = ct
            for pre in ("pd", "lam", "w"):
                for li, fl in ((1, 2048), (2, 4096), (3, 4 * N3[pre])):
                    t = fx.tile([128, fl], f8e4, name=f"sb_{pre}w8_{li}")
                    nc.gpsimd.dma_start(
                        out=t[:], in_=bass.AP(w8d[(pre, li)], 0, [[fl, 128], [1, fl]])
                    )
                    ws[(pre, li)] = t
            dcat = fx.tile([128, 2 * DD], fp32r, name="sb_dcat")
            nc.gpsimd.dma_start(
                out=dcat[:], in_=bass.AP(dcat_d, 0, [[2 * DD, 128], [1, 2 * DD]])
            )
            dcatT = fx.tile([128, 5 * 256], fp32r, name="sb_dcatT")
            nc.gpsimd.dma_start(
                out=dcatT[:], in_=bass.AP(dcatT_d, 0, [[5 * 256, 128], [1, 5 * 256]])
            )
            eye = fx.tile([128, 128], fp32r, name="sb_eye")
            nc.gpsimd.dma_start(
                out=eye[:], in_=bass.AP(eye_d, 0, [[128, 128], [1, 128]])
            )
            # dcatN = -dcat/c derived on-device instead of a 640KB upload
            dcatN = fx.tile([128, 2 * DD], fp32r, name="sb_dcatN")
            nc.scalar.mul(dcatN[:], dcat[:], -inv_c)

            # ---- per-tile geometry ----
            def geom(t):
                Nv = TILE_NV[t]
                return None, None, Nv, Nv + (Nv % 2), sum(TILE_NV[:t])

            def ufbase(t):
                return sum(2 * (nv + nv % 2) for nv in TILE_NV[:t])

            st = [dict() for _ in TILE_NV]   # per-tile live handles

            # ---- prologue pieces (overlappable) ----
            def p_unfold(t):
                _, _, Nv, N, tok0 = geom(t)
                uf = wk.tile([128, 2 * N], fp32r, name=f"uf{t}", tag="uf", bufs=2)
                uf8 = wk.tile([128, 2 * N], f8e4, name=f"uf8_{t}", tag="uf8", bufs=2)
                nc.sync.dma_start(
                    out=uf[:],
                    in_=bass.AP(ufall_d, ufbase(t), [[FT, 128], [1, 2 * N]]),
                )
                # uf8 = fp8((uf - 0.5) * SU), one cheap DVE pass
                nc.vector.tensor_scalar(
                    uf8[:], uf[:].bitcast(fp32), 0.5, SU, Alu.subtract, Alu.mult
                )
                st[t]["uf"] = uf
                st[t]["uf8"] = uf8
                st[t]["thr"] = wk.tile([128, 5 * N], fp32, name=f"thr{t}", tag="thr", bufs=2)
                st[t]["wg"] = wk.tile([128, 2 * N], bf16, name=f"wg{t}", tag="wg", bufs=2)

            def p_l1(t, pre):
                _, _, _, N, _ = geom(t)
                uf8 = st[t]["uf8"][:].rearrange("p (k n) -> p k n", k=2)
                ct = cs[pre]
                h18 = wk.tile([128, 8 * N], f8e4, name=f"h18_{t}_{pre}", tag="h18", bufs=2)
                w1v = ws[(pre, 1)][:].rearrange("p (j m) -> p j m", j=2)
                for g in range(8):
                    ps1 = pp.tile([128, N], fp32, name=f"ps1_{t}_{pre}_{g}", tag="ps", bufs=3)
                    nc.tensor.matmul(
                        ps1[:], w1v[:, :, g * 128 : g * 128 + 128], uf8,
                        start=True, stop=True, perf_mode=DR,
                    )
                    if pre == "w":
                        # uncentered: h18 = relu(psum*k1 + s1*b1c) directly (Scalar)
                        nc.scalar.activation(
                            h18[:, g * N : (g + 1) * N], ps1[:], Act.Relu,
                            bias=ct[:, CST_B1 + g : CST_B1 + g + 1],
                            scale=ct[:, CST_K[pre] : CST_K[pre] + 1],
                        )
                    else:
                        # fused relu+center+cast on Vector: (relu(ps+a1*b1c)-a1*mu1)*C2
                        nc.vector._custom_dve(
                            RELU_CENTER_ANT, out=h18[:, g * N : (g + 1) * N],
                            in0=ps1[:],
                            s0=ct[:, CST_B1 + g : CST_B1 + g + 1],
                            s1=ct[:, CST_MU1 + g : CST_MU1 + g + 1],
                            imm2=C2IMM,
                        )
                st[t][f"h18_{pre}"] = h18

            def p_l2(t, pre):
                _, _, _, N, _ = geom(t)
                h18 = st[t][f"h18_{pre}"][:].rearrange("p (k n) -> p k n", k=8)
                ct = cs[pre]
                h28 = wk.tile([128, 4 * N], f8e4, name=f"h28_{t}_{pre}", tag="h28", bufs=2)
                w2v = ws[(pre, 2)][:].rearrange("p (j2 j m) -> p j2 j m", j2=4, j=2)
                for g in range(4):
                    ps2 = pp.tile([128, N], fp32, name=f"ps2_{t}_{pre}_{g}", tag="ps", bufs=3)
                    for j2 in range(4):
                        nc.tensor.matmul(
                            ps2[:], w2v[:, j2, :, g * 128 : g * 128 + 128],
                            h18[:, 2 * j2 : 2 * j2 + 2, :],
                            start=(j2 == 0), stop=(j2 == 3), perf_mode=DR,
                        )
                    if pre == "w":
                        nc.scalar.activation(
                            h28[:, g * N : (g + 1) * N], ps2[:], Act.Relu,
                            bias=ct[:, CST_B2 + g : CST_B2 + g + 1],
                            scale=ct[:, CST_K[pre] + 1 : CST_K[pre] + 2],
                        )
                    else:
                        nc.vector._custom_dve(
                            RELU_CENTER_ANT, out=h28[:, g * N : (g + 1) * N],
                            in0=ps2[:],
                            s0=ct[:, CST_B2 + g : CST_B2 + g + 1],
                            s1=ct[:, CST_MU2 + g : CST_MU2 + g + 1],
                            imm2=C2IMM,
                        )
                st[t][f"h28_{pre}"] = h28

            def p_l3(t, pre):
                _, _, _, N, _ = geom(t)
                n3 = N3[pre]
                h28 = st[t][f"h28_{pre}"][:].rearrange("p (k n) -> p k n", k=4)
                ct = cs[pre]
                thr, wg = st[t]["thr"], st[t]["wg"]
                w3v = ws[(pre, 3)][:].rearrange("p (j2 j m) -> p j2 j m", j2=2, j=2)
                for mo in range(NB3[pre]):
                    sz = min(128, n3 - mo * 128)
                    ps3 = pp.tile([128, N], fp32, name=f"ps3_{t}_{pre}_{mo}", tag="ps", bufs=3)
                    for j2 in range(2):
                        nc.tensor.matmul(
                            ps3[0:sz], w3v[:, j2, :, mo * 128 : mo * 128 + sz],
                            h28[:, 2 * j2 : 2 * j2 + 2, :],
                            start=(j2 == 0), stop=(j2 == 1), perf_mode=DR,
                        )
                    kap = ct[:, CST_K[pre] + 2 : CST_K[pre] + 3]
                    if pre == "pd":
                        nc.scalar.activation(
                            thr[:, mo * N : (mo + 1) * N], ps3[:], Act.Identity,
                            bias=ct[:, CST_B3 + mo : CST_B3 + mo + 1], scale=kap,
                        )
                    elif pre == "lam":
                        nc.scalar.activation(
                            thr[0:112, 4 * N : 5 * N], ps3[0:112], Act.Identity,
                            bias=ct[0:112, CST_B3 : CST_B3 + 1], scale=kap[0:112],
                        )
                    else:
                        nc.scalar.activation(
                            wg[:, mo * N : (mo + 1) * N], ps3[:], Act.Sigmoid,
                            bias=ct[:, CST_B3 + mo : CST_B3 + mo + 1], scale=kap,
                        )

            def p_y(t):
                _, _, _, N, _ = geom(t)
                uf, thr = st[t]["uf"], st[t]["thr"]
                yc = wk.tile([128, 5 * N], fp32r, name=f"yc{t}", tag="yc", bufs=2)
                z = wk.tile([128, 5 * N], fp32r, name=f"z{t}_0", tag="z", bufs=2)
                for mc in range(5):
                    sz = D_SZ[mc]
                    d0 = 128 * mc
                    psy = pp.tile([128, N], fp32, name=f"psy_{t}_{mc}", tag="ps", bufs=3)
                    nc.tensor.matmul(
                        psy[0:sz], dcat[:, d0 : d0 + sz], uf[:, 0:N], start=True, stop=False
                    )
                    nc.tensor.matmul(
                        psy[0:sz], dcat[:, DD + d0 : DD + d0 + sz], uf[:, N : 2 * N],
                        start=False, stop=True,
                    )
                    nc.scalar.mul(yc[0:sz, mc * N : (mc + 1) * N], psy[0:sz], inv_c)
                    nc.vector._custom_dve(
                        SOFT_SHRINK_ANT,
                        out=z[0:sz, mc * N : (mc + 1) * N],
                        in0=psy[0:sz], in1=thr[0:sz, mc * N : (mc + 1) * N],
                    )
                st[t]["yc"] = yc
                st[t]["z"] = z

            def prologue_pieces(t):
                yield from (
                    lambda: p_unfold(t),
                    lambda: p_l1(t, "pd"), lambda: p_l2(t, "pd"), lambda: p_l3(t, "pd"),
                    lambda: p_l1(t, "lam"), lambda: p_l2(t, "lam"), lambda: p_l3(t, "lam"),
                    lambda: p_l1(t, "w"), lambda: p_l2(t, "w"), lambda: p_l3(t, "w"),
                    lambda: p_y(t),
                )

            def lista_iter_a(t, it):
                _, _, _, N, _ = geom(t)
                z = st[t]["z"]
                # G = Dcat @ z   [256, N]
                g = wk.tile([128, 2 * N], fp32r, name=f"g{t}_{it}", tag="g", bufs=2)
                for fc in range(2):
                    psg = pp.tile([128, N], fp32, name=f"psg_{t}_{it}_{fc}", tag="psg", bufs=2)
                    for kc in range(5):
                        szk = D_SZ[kc]
                        nc.tensor.matmul(
                            psg[:],
                            dcatT[0:szk, kc * 256 + fc * 128 : kc * 256 + fc * 128 + 128],
                            z[0:szk, kc * N : (kc + 1) * N],
                            start=(kc == 0), stop=(kc == 4),
                        )
                    nc.scalar.copy(g[:, fc * N : (fc + 1) * N], psg[:])
                st[t]["g"] = g

            def lista_iter_b(t, it):
                _, _, _, N, _ = geom(t)
                thr, yc, z = st[t]["thr"], st[t]["yc"], st[t]["z"]
                g = st[t]["g"]
                zn = wk.tile([128, 5 * N], fp32r, name=f"z{t}_{it + 1}", tag="z", bufs=2)
                for mc in range(5):
                    sz = D_SZ[mc]
                    d0 = 128 * mc
                    ps = pp.tile([128, N], fp32, name=f"psl_{t}_{it}_{mc}", tag="psl", bufs=3)
                    nc.tensor.matmul(
                        ps[0:sz], eye[0:sz, 0:sz], yc[0:sz, mc * N : (mc + 1) * N],
                        start=True, stop=False,
                    )
                    nc.tensor.matmul(
                        ps[0:sz], eye[0:sz, 0:sz], z[0:sz, mc * N : (mc + 1) * N],
                        start=False, stop=False,
                    )
                    for fc in range(2):
                        nc.tensor.matmul(
                            ps[0:sz],
                            dcatN[:, fc * DD + d0 : fc * DD + d0 + sz],
                            g[:, fc * N : (fc + 1) * N],
                            start=False, stop=(fc == 1),
                        )
                    nc.vector._custom_dve(
                        SOFT_SHRINK_ANT,
                        out=zn[0:sz, mc * N : (mc + 1) * N],
                        in0=ps[0:sz], in1=thr[0:sz, mc * N : (mc + 1) * N],
                    )
                st[t]["z"] = zn

            def emit_xp(t):
                _, _, Nv, N, tok0 = geom(t)
                z, wg = st[t]["z"], st[t]["wg"]
                for fc in range(2):
                    psx = pp.tile([128, N], fp32, name=f"psx_{t}_{fc}", tag="ps", bufs=3)
                    for kc in range(5):
                        szk = D_SZ[kc]
                        nc.tensor.matmul(
                            psx[:],
                            dcatT[0:szk, kc * 256 + fc * 128 : kc * 256 + fc * 128 + 128],
                            z[0:szk, kc * N : (kc + 1) * N],
                            start=(kc == 0), stop=(kc == 4),
                        )
                    nc.vector.tensor_scalar(psx[:], psx[:], 0.0, 1.0, Alu.max, Alu.min)
                    px = wk.tile([128, N], bf16, name=f"px{t}_{fc}", tag="px", bufs=2)
                    nc.vector.tensor_tensor(px[:], psx[:], wg[:, fc * N : (fc + 1) * N], Alu.mult)
                    nc.scalar.dma_start(
                        out=bass.AP(px_o, fc * 128 * LC + tok0, [[LC, 128], [1, Nv]]),
                        in_=px[:, 0:Nv],
                    )
                    nc.scalar.dma_start(
                        out=bass.AP(wg_o, fc * 128 * LC + tok0, [[LC, 128], [1, Nv]]),
                        in_=wg[:, fc * N : fc * N + Nv],
                    )

            # ---- driver: software-pipeline tiles; next-tile prologue pieces
            # are slotted at BOTH half-iteration boundaries so the PE always
            # has independent work while g-copies / softs drain ----
            n_tiles = len(TILE_NV)
            for piece in prologue_pieces(0):
                piece()
            n_slots = 2 * T_LISTA
            for t in range(n_tiles):
                nxt = list(prologue_pieces(t + 1)) if t + 1 < n_tiles else []
                slots = [nxt[(len(nxt) * i) // n_slots : (len(nxt) * (i + 1)) // n_slots]
                         for i in range(n_slots)]
                for it in range(T_LISTA):
                    lista_iter_a(t, it)
                    for piece in slots[2 * it]:
                        piece()
                    lista_iter_b(t, it)
                    for piece in slots[2 * it + 1]:
                        piece()
                emit_xp(t)

    nc.compile()
    return nc


# --------------------------------------------------------------------------
# host-side small ops (per sample): ext -> sd MLP -> CBAM -> Dcat
# --------------------------------------------------------------------------

def _host_sd(img2d, p, c_val):
    # ext: stride-8 unfold, every 2nd patch, first 112   [112, 256]
    ext = np.empty((112, 256), np.float32)
    for tt in range(112):
        ir, ic = divmod(2 * tt, 15)
        ext[tt] = img2d[8 * ir : 8 * ir + 16, 8 * ic : 8 * ic + 16].reshape(256)
    h = ext
    for wname, bname in (("s1w", "s1b"), ("s2w", "s2b"), ("s3w", "s3b")):
        h = np.maximum(h @ p[wname] + p[bname], 0.0, dtype=np.float32)
    sd = (h @ p["s4w"] + p["s4b"]).astype(np.float32)          # [112, 256]
    nrm = np.maximum(np.linalg.norm(sd, axis=-1, keepdims=True), 1e-12)
    sd = (sd / nrm).astype(np.float32)
    v = sd.T.reshape(256, 8, 14)                                # channels, 8x14
    def camlp(vec):
        return np.maximum(vec @ p["caw1"], 0.0) @ p["caw2"]
    ca = 1.0 / (1.0 + np.exp(-(camlp(v.mean(axis=(1, 2))) + camlp(v.max(axis=(1, 2))))))
    v = (v * ca[:, None, None]).astype(np.float32)
    s2 = np.stack([v.mean(axis=0), v.max(axis=0)])              # [2, 8, 14]
    pad = np.zeros((2, 14, 20), np.float32)
    pad[:, 3:11, 3:17] = s2
    sa = np.zeros((8, 14), np.float32)
    saw = p["saw"][0]                                           # [2,7,7]
    for ch in range(2):
        for dy in range(7):
            for dx in range(7):
                sa += saw[ch, dy, dx] * pad[ch, dy : dy + 8, dx : dx + 14]
    v = (v * (1.0 / (1.0 + np.exp(-sa)))[None]).astype(np.float32)
    sd = v.reshape(256, 112)
    dcat = np.concatenate([p["Dict"], sd], axis=1).astype(np.float32)   # [256, 624]
    return ext, dcat


def _w8_blob(key, W1, W2, W3, n3, q1, q2, q3):
    """Per-layer fp8 weight blobs with pre-scales q1/q2/q3 applied."""
    if key in _WBLOB_CACHE:
        return _WBLOB_CACHE[key]
    a1 = np.ascontiguousarray(
        (W1 * q1).reshape(2, 128, 1024).transpose(1, 0, 2).reshape(128, 2048)).astype(F8NP)
    a2 = np.ascontiguousarray(
        (W2 * q2).reshape(8, 128, 512).transpose(1, 0, 2).reshape(128, 4096)).astype(F8NP)
    a3 = np.ascontiguousarray(
        (W3 * q3).reshape(4, 128, n3).transpose(1, 0, 2).reshape(128, 4 * n3)).astype(F8NP)
    _WBLOB_CACHE[key] = (a1, a2, a3)
    return _WBLOB_CACHE[key]


def _cols(v, ng):
    """[ng*128] (or shorter, zero-padded) -> [128, ng] column layout."""
    out = np.zeros(ng * 128, np.float32)
    out[: v.shape[0]] = v
    return out.reshape(ng, 128).T


def _wmax(W):
    return float(np.abs(W).max()) + 1e-30


def _mlp_quant(src, ext, W1, b1, W2, b2, W3, b3, n3, nb3, cdiv, centered):
    """Returns (w8 blob, cst [128,ncol]) for one MLP on one sample."""
    h1e = np.maximum(ext @ W1 + b1, 0.0)
    h2e = np.maximum(h1e @ W2 + b2, 0.0)
    if centered:
        mu1 = h1e.mean(0).astype(np.float32)
        mu2 = h2e.mean(0).astype(np.float32)
    else:
        mu1 = np.zeros_like(b1)
        mu2 = np.zeros_like(b2)
    a1 = float(np.abs(h1e - mu1).max())
    a2 = float(np.abs(h2e - mu2).max())
    s1 = min(_p2(240.0 / (16.0 * (a1 + 1e-6))), 2.0 ** 16)
    s2 = min(_p2(240.0 / (16.0 * (a2 + 1e-6))), 2.0 ** 16)
    q3 = _p2(240.0 / (2.0 * _wmax(W3)))
    if centered:
        # h8 = (relu(psum + a*b) - a*mu) * C2IMM on the DVE; alpha = SU*q1 etc.
        q1 = min(s1 / (SU * C2IMM), _p2(240.0 / (2.0 * _wmax(W1))))
        q2 = min(s2 / (s1 * C2IMM), _p2(240.0 / (2.0 * _wmax(W2))))
        al1 = SU * q1
        al2 = s1 * q2
        kvec = [0.0, 0.0, 1.0 / (s1 * q2 * C2IMM * q3 * cdiv)]
        b1s = al1 * (b1 + 0.5 * W1.sum(0))
        mu1s = al1 * mu1
        b2s = al2 * (b2 + mu1 @ W2)
        mu2s = al2 * mu2
    else:
        # h8 = relu(psum*k + s*b) via Scalar ACT; free choice of q1/q2
        q1 = _p2(240.0 / (2.0 * _wmax(W1)))
        q2 = _p2(240.0 / (2.0 * _wmax(W2)))
        kvec = [s1 / (SU * q1), s2 / (s1 * q2), 1.0 / (s2 * q3 * cdiv)]
        b1s = s1 * (b1 + 0.5 * W1.sum(0))
        mu1s = np.zeros_like(b1)
        b2s = s2 * b2
        mu2s = np.zeros_like(b2)
    b3s = ((b3 + mu2 @ W3) / cdiv).astype(np.float32)
    blob = _w8_blob((src, q1, q2, q3), W1, W2, W3, n3, q1, q2, q3)
    ncol = 24 + nb3 + 3
    cst = np.zeros((128, ncol), np.float32)
    cst[:, CST_B1 : CST_B1 + 8] = _cols(b1s.astype(np.float32), 8)
    cst[:, CST_MU1 : CST_MU1 + 8] = _cols(mu1s.astype(np.float32), 8)
    cst[:, CST_B2 : CST_B2 + 4] = _cols(b2s.astype(np.float32), 4)
    cst[:, CST_MU2 : CST_MU2 + 4] = _cols(mu2s.astype(np.float32), 4)
    cst[:, CST_B3 : CST_B3 + nb3] = _cols(b3s, nb3)
    cst[:, 24 + nb3] = kvec[0]
    cst[:, 24 + nb3 + 1] = kvec[1]
    cst[:, 24 + nb3 + 2] = kvec[2]
    return blob, cst


def _fold(pfull):
    # pfull [256, 12769] feature-major -> overlap-add [128,128]
    out = np.zeros((128, 128), np.float32)
    pr = pfull.reshape(16, 16, PR, PR)
    for kh in range(16):
        for kw in range(16):
            out[kh : kh + PR, kw : kw + PR] += pr[kh, kw]
    return out


def _assemble(chunks):
    # chunks: list of 4 arrays [256, 3193] -> [256, 12769]
    full = np.empty((256, L_FULL), np.float32)
    for q in range(4):
        full[:, T0S[q] : T0S[q] + LC] = chunks[q]
    return full


def _unfold_full(img):
    """[256, 12769] feature-major unfold of one [128,128] image."""
    sw = np.lib.stride_tricks.sliding_window_view(img, (16, 16))
    return sw.transpose(2, 3, 0, 1).reshape(256, L_FULL)


def _build_ufall(uf_full, t0):
    """Per-core pre-unfolded uf blob [128, sum(2*N_t)] from token t0."""
    cols = []
    ofs = 0
    for Nv in TILE_NV:
        N = Nv + (Nv % 2)
        blk = uf_full[:, t0 + ofs : t0 + ofs + Nv]
        if N > Nv:
            blk = np.concatenate([blk, np.full((256, N - Nv), 0.5, np.float32)], axis=1)
        cols.append(blk.reshape(2, 128, N).transpose(1, 0, 2).reshape(128, 2 * N))
        ofs += Nv
    return np.ascontiguousarray(np.concatenate(cols, axis=1), dtype=np.float32)


# --------------------------------------------------------------------------
# stage driver
# --------------------------------------------------------------------------

def _run_stage(nc, imgs, p, lam_pre, pd_pre, c_val, results_holder=None, trace=False, tmpdir=None):
    eye_b = np.eye(128, dtype=np.float32)
    per_sample = []
    uf_fulls = []
    for n in range(2):
        uf_fulls.append(_unfold_full(imgs[n]))
        ext, dcat = _host_sd(imgs[n], p, c_val)
        quants = {}
        for dev_pre, src, cdiv in (("pd", pd_pre, c_val), ("lam", lam_pre, c_val), ("w", "w", 1.0)):
            quants[dev_pre] = _mlp_quant(
                src, ext, p[src + "1w"], p[src + "1b"], p[src + "2w"], p[src + "2b"],
                p[src + "3w"], p[src + "3b"], N3[dev_pre], NB3[dev_pre], cdiv,
                centered=(dev_pre != "w"),
            )
        dcat_b = dcat.reshape(2, 128, DD).transpose(1, 0, 2).reshape(128, 2 * DD)
        dT = np.zeros((640, 256), np.float32)
        dT[:DD] = dcat.T
        dcatT_b = dT.reshape(5, 128, 256).transpose(1, 0, 2).reshape(128, 5 * 256)
        per_sample.append((quants, np.ascontiguousarray(dcat_b),
                           np.ascontiguousarray(dcatT_b)))

    in_maps = []
    for core in range(8):
        n, q = divmod(core, 4)
        quants, dcat_b, dcatT_b = per_sample[n]
        m = {}
        m["ufall"] = _build_ufall(uf_fulls[n], T0S[q])
        for pre in ("pd", "lam", "w"):
            for li in (1, 2, 3):
                m[f"{pre}w8_{li}"] = quants[pre][0][li - 1]
            m[f"{pre}cst"] = quants[pre][1]
        m["dcat_b"] = dcat_b
        m["dcatT_b"] = dcatT_b
        m["eye_b"] = eye_b
        in_maps.append(m)

    import time as _time
    last = None
    for _attempt in range(4):
        try:
            res = run_bass_kernel_spmd(nc, in_maps, list(range(8)), trace=trace, tmpdir=tmpdir)
            break
        except Exception as e:  # transient NRT device errors: retry after backoff
            last = e
            _time.sleep(5.0 + 10.0 * _attempt)
    else:
        raise last
    if results_holder is not None:
        results_holder.append(res)

    out = np.empty((2, 128, 128), np.float32)
    for n in range(2):
        px = _assemble([np.asarray(res.results[4 * n + q]["px_o"]).astype(np.float32)
                        for q in range(4)])
        wgf = _assemble([np.asarray(res.results[4 * n + q]["wg_o"]).astype(np.float32)
                        for q in range(4)])
        num = _fold(px)
        den = _fold(wgf)
        out[n] = num / den
    return out


def kernel(**inputs) -> np.ndarray:
    p = {k: np.asarray(v, np.float32) for k, v in inputs.items()}
    c_val = float(np.asarray(inputs["c"]))
    key = ("nc", c_val)
    if key not in _NC_CACHE:
        _NC_CACHE[key] = _build(c_val)
    nc = _NC_CACHE[key]
    x = p["x"]  # [2,1,128,128]
    imgs1 = [x[n, 0] for n in range(2)]
    res1 = _run_stage(nc, imgs1, p, "a", "p", c_val)
    imgs2 = [res1[n] for n in range(2)]
    res2 = _run_stage(nc, imgs2, p, "b", "q", c_val)
    return res2.reshape(2, 1, 128, 128).astype(np.float32)
